# revision 20
# baseline (speedup 1.0000x reference)
"""Multi-head attention (B=2, T=2048, D=2048, 16 heads) on 8 NeuronCores.

Sharding: DP=2 over batch x TP=4 over heads (4 heads/core).
Core c handles batch b=c//4, head group r=c%4 (heads 4r..4r+3).

v4 design (bf16 projections, fp16 attention-probability path):
  P1a: V projection (tokens on partitions, fp16), then K^T for all
       heads (fused [128,1024] PSUM tiles).
  P1c: per head h: Q^T(h), then attention(h): S^T chunks with shared K
       stationary, exp on ScalarE (fp16), fp16 DVE row-chunk
       accumulation, PV with shared V stationary, denominator broadcast
       via all-ones matmul + reciprocal_approx_fast, per-(h,half) bf16
       AllGather fired immediately.  The exp stream of head h hides
       under the Q projection of head h+1.
  P3A: partial output projection over heads 0-2 (12 of 16 contraction
       chunks), emitted after the head loop but scheduled under the
       ACT-bound final attention head; partials parked in SBUF as bf16.
  P3B: head-3 chunks after the last AllGather, combined with the
       partials on DVE, fp32 out.

Output per core: out^T (512 out-cols, 2048 tokens) fp32; host transposes
and concatenates.
"""

import math

import numpy as np
import ml_dtypes

import concourse.bass as bass
import concourse.mybir as mybir
import concourse.tile as tile
from concourse import bacc
from concourse.bass_utils import run_bass_kernel_spmd

D = 2048
T = 2048
HG = 4  # heads per core
DH = 128
NI = 16  # contraction chunks of 128 over D
NT = 16  # token chunks of 128
SCALE = 1.0 / math.sqrt(DH)
F32 = mybir.dt.float32
F32R = mybir.dt.float32r
BF16 = mybir.dt.bfloat16
FP16 = mybir.dt.float16
GROUPS = [[0, 1, 2, 3], [4, 5, 6, 7]]

_CACHED = {}


def build():
    nc = bacc.Bacc("TRN2", target_bir_lowering=False, debug=False, num_devices=8)
    xT = nc.declare_dram_parameter("xT", [D, T], BF16, isOutput=False)
    wqT = nc.declare_dram_parameter("wqT", [D, HG * DH], BF16, isOutput=False)
    wkT = nc.declare_dram_parameter("wkT", [D, HG * DH], BF16, isOutput=False)
    wvT = nc.declare_dram_parameter("wvT", [D, HG * DH], BF16, isOutput=False)
    woT = nc.declare_dram_parameter("woT", [D, HG * DH], BF16, isOutput=False)
    out = nc.declare_dram_parameter("out", [HG * DH, T], F32, isOutput=True)

    with tile.TileContext(nc) as tc:
        with (
            tc.tile_pool(name="dram", bufs=1, space="DRAM") as dram,
            tc.tile_pool(name="keep", bufs=1) as keep,
        ):
            attn_mine = [
                [
                    dram.tile(
                        [DH, 1024], BF16, name=f"am{h}_{qp}", tag=f"am{h}_{qp}"
                    )
                    for qp in range(2)
                ]
                for h in range(HG)
            ]
            attn_all = [
                [
                    dram.tile(
                        [4 * DH, 1024], BF16, name=f"aa{h}_{qp}", tag=f"aa{h}_{qp}"
                    )
                    for qp in range(2)
                ]
                for h in range(HG)
            ]

            # whole-kernel residents
            qT_sb = keep.tile([128, HG, T], BF16)  # Q^T: [dh, head, tok]
            kT_sb = keep.tile([128, HG, T], BF16)
            v_sb = keep.tile([128, NT, HG * DH], FP16)  # V: [tok128, tchunk, hdims]
            wo_sb = keep.tile([128, NI, HG * DH], BF16)
            o_part = keep.tile([128, 2, 4, 1024], BF16)  # P3A partials [qp][cb]
            ones_f32 = keep.tile([128, 128], F32)
            nc.vector.memset(ones_f32[:], 1.0)
            ones_sb = keep.tile([128, 128], FP16)
            nc.vector.tensor_copy(ones_sb[:], ones_f32[:])

            with tc.tile_pool(name="p1x", bufs=1) as p1x:
                x_sb = p1x.tile([128, NI, T], BF16)  # x^T resident: 64KB/part

                # ---------------- Phase 1a: V projection ----------------
                with (
                    tc.tile_pool(name="p1wvk", bufs=2) as p1wvk,
                    tc.tile_pool(name="p1pv", bufs=2, space="PSUM") as p1pv,
                    tc.tile_pool(name="p1pk", bufs=2, space="PSUM") as p1pk,
                ):
                    # DMA order: wv, wk, x token-half 0, x half 1 — V and
                    # K(tp0) chains both consume each arriving x chunk, so
                    # the PE saturates during the x load (~11us in).
                    wv_sb = p1wvk.tile([128, NI, HG * DH], BF16, tag="w_sb")
                    for i in range(NI):
                        nc.sync.dma_start(
                            out=wv_sb[:, i, :], in_=wvT[i * 128 : (i + 1) * 128, :]
                        )
                    wk_sb = p1wvk.tile([128, NI, HG * DH], BF16, tag="w_sb")
                    for i in range(NI):
                        nc.sync.dma_start(
                            out=wk_sb[:, i, :], in_=wkT[i * 128 : (i + 1) * 128, :]
                        )
                    for i in range(NI):
                        nc.sync.dma_start(
                            out=x_sb[:, i, 0:1024],
                            in_=xT[i * 128 : (i + 1) * 128, 0:1024],
                        )
                    for i in range(NI):
                        nc.sync.dma_start(
                            out=x_sb[:, i, 1024:2048],
                            in_=xT[i * 128 : (i + 1) * 128, 1024:2048],
                        )

                    def v_block(tc_lo, tc_hi):
                        for tc_i in range(tc_lo, tc_hi):
                            ps = p1pv.tile([128, 512], F32, tag="v_ps")
                            for i in range(NI):
                                nc.tensor.matmul(
                                    ps[:],
                                    x_sb[:, i, tc_i * 128 : (tc_i + 1) * 128],
                                    wv_sb[:, i, :],
                                    start=(i == 0),
                                    stop=(i == NI - 1),
                                )
                            nc.vector.tensor_copy(v_sb[:, tc_i, :], ps[:])

                    def k_block(tp):
                        for m in range(HG):
                            ps = p1pk.tile([128, 1024], F32, tag="k_ps")
                            for i in range(NI):
                                lhsT = wk_sb[:, i, m * 128 : (m + 1) * 128]
                                for j in range(2):
                                    nc.tensor.matmul(
                                        ps[:, j * 512 : (j + 1) * 512],
                                        lhsT,
                                        x_sb[
                                            :,
                                            i,
                                            tp * 1024
                                            + j * 512 : tp * 1024
                                            + (j + 1) * 512,
                                        ],
                                        start=(i == 0),
                                        stop=(i == NI - 1),
                                    )
                            nc.vector.tensor_copy(
                                kT_sb[:, m, tp * 1024 : (tp + 1) * 1024], ps[:]
                            )

                    v_block(0, 8)  # token half 0
                    k_block(0)
                    v_block(8, NT)  # token half 1
                    k_block(1)

                # -------- Phase 1c + 2: Q^T(h) then attention(h) --------
                with (
                    tc.tile_pool(name="p1pq", bufs=1, space="PSUM") as p1pq,
                    tc.tile_pool(name="p2e", bufs=4) as p2e,
                    tc.tile_pool(name="p2acc", bufs=2) as p2acc,
                    tc.tile_pool(name="p2n", bufs=2) as p2n,
                    tc.tile_pool(name="p2at", bufs=1) as p2at,
                    tc.tile_pool(name="p2ps", bufs=2, space="PSUM") as p2ps,
                    tc.tile_pool(name="p2pa", bufs=1, space="PSUM") as p2pa,
                ):
                    with tc.tile_pool(name="p1wq", bufs=1) as p1wq:
                        wq_sb = p1wq.tile([128, NI, HG * DH], BF16)
                        for i in range(NI):
                            nc.sync.dma_start(
                                out=wq_sb[:, i, :],
                                in_=wqT[i * 128 : (i + 1) * 128, :],
                            )
                        for i in range(NI):
                            nc.sync.dma_start(
                                out=wo_sb[:, i, :],
                                in_=woT[i * 128 : (i + 1) * 128, :],
                            )

                        for h in range(HG):
                            # Q^T for head h
                            for tp in range(2):
                                ps = p1pq.tile([128, 1024], F32, tag="q_ps")
                                for i in range(NI):
                                    lhsT = wq_sb[:, i, h * 128 : (h + 1) * 128]
                                    for j in range(2):
                                        nc.tensor.matmul(
                                            ps[:, j * 512 : (j + 1) * 512],
                                            lhsT,
                                            x_sb[
                                                :,
                                                i,
                                                tp * 1024
                                                + j * 512 : tp * 1024
                                                + (j + 1) * 512,
                                            ],
                                            start=(i == 0),
                                            stop=(i == NI - 1),
                                        )
                                nc.vector.tensor_copy(
                                    qT_sb[:, h, tp * 1024 : (tp + 1) * 1024], ps[:]
                                )

                            # attention for head h, over two 1024-wide halves
                            for qp in range(2):
                                acc = p2acc.tile([128, 1024], FP16, tag="acc")
                                attn_ps = p2pa.tile([128, 1024], F32, tag="attn_ps")
                                q0 = qp * 1024
                                for k in range(NT):
                                    s_ps = p2ps.tile([128, 1024], F32, tag="s_ps")
                                    kh = kT_sb[:, h, k * 128 : (k + 1) * 128]
                                    for j in range(2):
                                        nc.tensor.matmul(
                                            s_ps[:, j * 512 : (j + 1) * 512],
                                            kh,
                                            qT_sb[
                                                :,
                                                h,
                                                q0 + j * 512 : q0 + (j + 1) * 512,
                                            ],
                                        )
                                    expS = p2e.tile([128, 1024], FP16, tag="expS")
                                    nc.scalar.activation(
                                        expS[:],
                                        s_ps[:],
                                        mybir.ActivationFunctionType.Exp,
                                        scale=SCALE,
                                    )
                                    if k == 0:
                                        nc.vector.tensor_copy(acc[:], expS[:])
                                    else:
                                        nc.vector.tensor_add(
                                            acc[:], acc[:], expS[:]
                                        )
                                    vh = v_sb[:, k, h * 128 : (h + 1) * 128]
                                    for j in range(2):
                                        nc.tensor.matmul(
                                            attn_ps[:, j * 512 : (j + 1) * 512],
                                            vh,
                                            expS[:, j * 512 : (j + 1) * 512],
                                            start=(k == 0),
                                            stop=(k == NT - 1),
                                        )
                                # denominator broadcast via ones-MM
                                bcsum = p2ps.tile([128, 1024], F32, tag="s_ps")
                                for j in range(2):
                                    nc.tensor.matmul(
                                        bcsum[:, j * 512 : (j + 1) * 512],
                                        ones_sb[:],
                                        acc[:, j * 512 : (j + 1) * 512],
                                    )
                                recip = p2n.tile([128, 1024], F32, tag="recip")
                                nc.vector.reciprocal_approx_fast(
                                    out=recip[:], in_=bcsum[:]
                                )
                                attn_sb = p2at.tile(
                                    [128, 1024], BF16, tag="attn_sb"
                                )
                                nc.vector.tensor_mul(
                                    attn_sb[:], attn_ps[:], recip[:]
                                )
                                nc.sync.dma_start(
                                    out=attn_mine[h][qp][:], in_=attn_sb[:]
                                )
                                nc.gpsimd.collective_compute(
                                    "AllGather",
                                    mybir.AluOpType.bypass,
                                    replica_groups=GROUPS,
                                    ins=[attn_mine[h][qp].opt()],
                                    outs=[attn_all[h][qp].opt()],
                                )

                    # ------- Phase 3A: partial out^T over heads 0-2 -------
                    # (scheduled under the ACT-bound final attention head)
                    with tc.tile_pool(name="p3at", bufs=12) as p3at:
                        for qp in range(2):
                            a_tiles = {}
                            for h in range(HG - 1):
                                for r in range(4):
                                    at = p3at.tile(
                                        [128, 1024], BF16, name="a_t", tag="a_t"
                                    )
                                    nc.sync.dma_start(
                                        out=at[:],
                                        in_=attn_all[h][qp][
                                            r * 128 : (r + 1) * 128, :
                                        ],
                                    )
                                    a_tiles[(h, r)] = at
                            for cb in range(4):
                                ps = p1pq.tile([128, 1024], F32, tag="q_ps")
                                n_ch = 0
                                for h in range(HG - 1):
                                    for r in range(4):
                                        g = r * HG + h
                                        lhsT = wo_sb[
                                            :, g, cb * 128 : (cb + 1) * 128
                                        ]
                                        for j in range(2):
                                            nc.tensor.matmul(
                                                ps[:, j * 512 : (j + 1) * 512],
                                                lhsT,
                                                a_tiles[(h, r)][
                                                    :, j * 512 : (j + 1) * 512
                                                ],
                                                start=(n_ch == 0),
                                                stop=(n_ch == 4 * (HG - 1) - 1),
                                            )
                                        n_ch += 1
                                nc.vector.tensor_copy(o_part[:, qp, cb, :], ps[:])

            # ------- Phase 3B: head-3 chunks + combine -------
            with (
                tc.tile_pool(name="p3b", bufs=8) as p3b,
                tc.tile_pool(name="p3o", bufs=3) as p3o,
                tc.tile_pool(name="p3pb", bufs=2, space="PSUM") as p3pb,
            ):
                h = HG - 1
                for qp in range(2):
                    a_tiles = []
                    for r in range(4):
                        at = p3b.tile([128, 1024], BF16, name="a_b", tag="a_b")
                        nc.sync.dma_start(
                            out=at[:],
                            in_=attn_all[h][qp][r * 128 : (r + 1) * 128, :],
                        )
                        a_tiles.append(at)
                    for cb in range(4):
                        ps = p3pb.tile([128, 1024], F32, tag="b_ps")
                        for r in range(4):
                            g = r * HG + h
                            lhsT = wo_sb[:, g, cb * 128 : (cb + 1) * 128]
                            for j in range(2):
                                nc.tensor.matmul(
                                    ps[:, j * 512 : (j + 1) * 512],
                                    lhsT,
                                    a_tiles[r][:, j * 512 : (j + 1) * 512],
                                    start=(r == 0),
                                    stop=(r == 3),
                                )
                        o_sb = p3o.tile([128, 1024], F32, tag="o_sb")
                        nc.vector.tensor_add(o_sb[:], ps[:], o_part[:, qp, cb, :])
                        nc.sync.dma_start(
                            out=out[
                                cb * 128 : (cb + 1) * 128,
                                qp * 1024 : (qp + 1) * 1024,
                            ],
                            in_=o_sb[:],
                        )

    nc.compile()
    return nc


def _get_nc():
    if "nc" not in _CACHED:
        _CACHED["nc"] = build()
    return _CACHED["nc"]


def kernel(x, Wq, Wk, Wv, Wo, _trace=False):
    x = np.asarray(x, dtype=np.float32)
    B = x.shape[0]
    bf = ml_dtypes.bfloat16

    xT_b = [np.ascontiguousarray(x[b].T).astype(bf) for b in range(B)]
    w_slices = []
    for r in range(4):
        sl = slice(r * 512, (r + 1) * 512)
        w_slices.append(
            {
                "wqT": np.ascontiguousarray(np.asarray(Wq)[sl, :].T).astype(bf),
                "wkT": np.ascontiguousarray(np.asarray(Wk)[sl, :].T).astype(bf),
                "wvT": np.ascontiguousarray(np.asarray(Wv)[sl, :].T).astype(bf),
                "woT": np.ascontiguousarray(np.asarray(Wo)[sl, :].T).astype(bf),
            }
        )

    in_maps = []
    for c in range(8):
        b, r = divmod(c, 4)
        in_maps.append({"xT": xT_b[b], **w_slices[r]})

    nc = _get_nc()
    res = run_bass_kernel_spmd(nc, in_maps, list(range(8)), trace=_trace)
    _CACHED["last_result"] = res

    out = np.empty((B, T, D), dtype=np.float32)
    for c in range(8):
        b, r = divmod(c, 4)
        out[b, :, r * 512 : (r + 1) * 512] = res.results[c]["out"].T
    return out
